# revision 1
# baseline (speedup 1.0000x reference)
"""Multi-head attention (B=2, S=2048, D=1024, H=16) on 8 Trainium2 NeuronCores.

Sharding: data-parallel over batch (2 groups of 4 cores) x tensor-parallel over
heads (4 heads per core). Each core:
  - computes qT/kT = (x @ Wqk_c).T via W.T @ x.T   (transposed layout, [ch, S])
  - computes v' = x @ Wv_c augmented with a ones column per head (for sumexp)
  - flash-style attention per head with exp on ScalarE (no max subtraction
    needed: scores ~ N(0,1)); mask folded in as additive bias per k-position
  - partial output projection over its 4 heads' channels -> [S, D]
Host sums the 4 partials per batch and adds b_out.

Matmuls run in float32r (full-rate fp32, ~1.5e-4 relative rounding).
All matmul operands are base-partition-0 (base-64 fp32-family matmuls hang
the HW); per-head K=64 contractions are zero-padded to K=128 instead.
"""
import os
import numpy as np

import concourse.bass as bass
from concourse import bacc
import concourse.mybir as mybir
import concourse.tile as tile
from concourse.bass_utils import run_bass_kernel_spmd

F32 = mybir.dt.float32
F32R = mybir.dt.float32r

B = 2
S = 2048
D = 1024
NH = 16
HD = 64
NH_LOC = 4            # heads per core
N_CORES = 8
KC = D // 128         # 8 contraction chunks for projections
ST = S // 128         # 16 sequence tiles of 128
QC = S // 512         # 4 q chunks of 512
CV = NH_LOC * (HD + 1)  # 260: v' channels (64 v + 1 ones) x 4 heads

_CACHED = {}


def _build_module():
    nc = bacc.Bacc()
    xt = nc.declare_dram_parameter("xt", [D, S], F32, isOutput=False)
    wqk = nc.declare_dram_parameter("wqk", [D, 512], F32, isOutput=False)
    bqk = nc.declare_dram_parameter("bqk", [128, 4], F32, isOutput=False)
    wv = nc.declare_dram_parameter("wv", [D, CV], F32, isOutput=False)
    bv = nc.declare_dram_parameter("bv", [128, CV], F32, isOutput=False)
    wout = nc.declare_dram_parameter("wout", [2 * 128, D], F32, isOutput=False)
    maskb = nc.declare_dram_parameter("maskb", [128, ST], F32, isOutput=False)
    out = nc.declare_dram_parameter("out", [S, D], F32, isOutput=True)

    with tile.TileContext(nc) as tc:
        with tc.tile_pool(name="persist", bufs=1) as persist, \
             tc.tile_pool(name="epool", bufs=3) as epool, \
             tc.tile_pool(name="spool", bufs=4) as spool, \
             tc.tile_pool(name="rpool", bufs=2) as rpool, \
             tc.tile_pool(name="mm", bufs=2, space="PSUM") as mm_ps, \
             tc.tile_pool(name="st", bufs=2, space="PSUM") as st_ps, \
             tc.tile_pool(name="vals", bufs=2, space="PSUM") as vals_ps:

            # ---- resident inputs ----
            xt_sb = persist.tile([128, KC, S], F32R)
            wqk_sb = persist.tile([128, KC, 512], F32R)
            wv_sb = persist.tile([128, KC, CV], F32R)
            wout_sb = persist.tile([128, 2, D], F32R)
            bqk_sb = persist.tile([128, 4], F32)
            bv_sb = persist.tile([128, CV], F32)
            maskb_sb = persist.tile([128, ST], F32)
            ones_sb = persist.tile([1, 64], F32)
            for kc in range(KC):
                nc.sync.dma_start(out=xt_sb[:, kc, :],
                                  in_=xt[kc * 128:(kc + 1) * 128, :].bitcast(F32R))
                nc.sync.dma_start(out=wqk_sb[:, kc, :],
                                  in_=wqk[kc * 128:(kc + 1) * 128, :].bitcast(F32R))
                nc.sync.dma_start(out=wv_sb[:, kc, :],
                                  in_=wv[kc * 128:(kc + 1) * 128, :].bitcast(F32R))
            nc.sync.dma_start(out=wout_sb[:, 0, :], in_=wout[0:128, :].bitcast(F32R))
            nc.sync.dma_start(out=wout_sb[:, 1, :], in_=wout[128:256, :].bitcast(F32R))
            nc.sync.dma_start(out=bqk_sb, in_=bqk[:, :])
            nc.sync.dma_start(out=bv_sb, in_=bv[:, :])
            nc.sync.dma_start(out=maskb_sb, in_=maskb[:, :])
            nc.vector.memset(ones_sb, 1.0)

            # ---- projected tensors ----
            qt_sb = persist.tile([128, 2, S], F32R)   # pair-packed q: pair hp rows 0:64=h(2hp),64:128=h(2hp+1)
            kt_sb = persist.tile([128, 4, S], F32R)   # per-head zero-padded k
            v_sb = persist.tile([128, ST, CV], F32R)  # v' natural layout per 128-seq tile
            valst_sb = persist.tile([128, 2, S], F32R)  # attention out channels x S

            # zero the padding halves of kt (head j occupies rows (j%2)*64..+64)
            nc.gpsimd.memset(kt_sb[64:128, 0, :].bitcast(mybir.dt.uint32), 0)
            nc.gpsimd.memset(kt_sb[0:64, 1, :].bitcast(mybir.dt.uint32), 0)
            nc.gpsimd.memset(kt_sb[64:128, 2, :].bitcast(mybir.dt.uint32), 0)
            nc.gpsimd.memset(kt_sb[0:64, 3, :].bitcast(mybir.dt.uint32), 0)

            # ---- phase 1: qT/kT projection (wqk.T @ xT) ----
            # m: 0 -> q pair 0, 1 -> q pair 1, 2 -> k heads 0/1, 3 -> k heads 2/3
            for m in (0, 2, 1, 3):
                for qc in range(QC):
                    ps = mm_ps.tile([128, 512], F32, tag="mm", name="ps_qk")
                    for kc in range(KC):
                        nc.tensor.matmul(
                            ps,
                            wqk_sb[:, kc, m * 128:(m + 1) * 128],
                            xt_sb[:, kc, qc * 512:(qc + 1) * 512],
                            start=(kc == 0), stop=(kc == KC - 1))
                    qs = slice(qc * 512, (qc + 1) * 512)
                    if m < 2:
                        nc.vector.tensor_scalar_add(
                            qt_sb[:, m, qs], ps, bqk_sb[:, m:m + 1])
                    else:
                        j0, j1 = 2 * (m - 2), 2 * (m - 2) + 1
                        nc.vector.tensor_scalar_add(
                            kt_sb[0:64, j0, qs], ps[0:64, :], bqk_sb[0:64, m:m + 1])
                        nc.vector.tensor_scalar_add(
                            kt_sb[64:128, j1, qs], ps[64:128, :], bqk_sb[64:128, m:m + 1])

            # ---- phase 2: v' projection (xT.T @ wv) ----
            for st in range(ST):
                ps = mm_ps.tile([128, CV], F32, tag="mm", name="ps_v")
                for kc in range(KC):
                    nc.tensor.matmul(
                        ps,
                        xt_sb[:, kc, st * 128:(st + 1) * 128],
                        wv_sb[:, kc, :],
                        start=(kc == 0), stop=(kc == KC - 1))
                nc.vector.tensor_add(v_sb[:, st, :], ps, bv_sb)

            # ---- phase 3: attention ----
            for hp in range(2):
                j0, j1 = 2 * hp, 2 * hp + 1
                for qc in range(QC):
                    qs = slice(qc * 512, (qc + 1) * 512)
                    va = vals_ps.tile([128, 512], F32, tag="vals", name="va")
                    vb = vals_ps.tile([128, 512], F32, tag="vals", name="vb")
                    for kc in range(ST):
                        ks = slice(kc * 128, (kc + 1) * 128)
                        stp = st_ps.tile([128, 2, 512], F32, tag="st", name="stp")
                        nc.tensor.matmul(stp[:, 0, :], kt_sb[:, j0, ks],
                                         qt_sb[:, hp, qs], start=True, stop=True)
                        nc.tensor.matmul(stp[:, 1, :], kt_sb[:, j1, ks],
                                         qt_sb[:, hp, qs], start=True, stop=True)
                        e_sb = epool.tile([128, 2, 512], F32R, tag="e", name="e_sb")
                        nc.scalar.activation(
                            out=e_sb, in_=stp,
                            func=mybir.ActivationFunctionType.Exp,
                            bias=maskb_sb[:, kc:kc + 1], scale=0.125)
                        nc.tensor.matmul(
                            va[0:65, :], v_sb[:, kc, j0 * 65:j0 * 65 + 65],
                            e_sb[:, 0, :], start=(kc == 0), stop=(kc == ST - 1))
                        nc.tensor.matmul(
                            vb[0:65, :], v_sb[:, kc, j1 * 65:j1 * 65 + 65],
                            e_sb[:, 1, :], start=(kc == 0), stop=(kc == ST - 1))
                    for j, va_ps in ((j0, va), (j1, vb)):
                        r1 = rpool.tile([1, 512], F32, tag="r1", name="r1")
                        nc.vector.reciprocal(r1, va_ps[64:65, :])
                        bc = mm_ps.tile([64, 512], F32, tag="mm", name="bc")
                        nc.tensor.matmul(bc, ones_sb, r1, start=True, stop=True)
                        rb = rpool.tile([64, 512], F32, tag="rb", name="rb")
                        nc.vector.tensor_copy(rb, bc)
                        nc.vector.tensor_mul(
                            valst_sb[(j % 2) * 64:(j % 2) * 64 + 64, j // 2, qs],
                            va_ps[0:64, :], rb)

            # ---- phase 4: output projection (valst.T @ wout) ----
            for sti in range(ST):
                ss = slice(sti * 128, (sti + 1) * 128)
                for dc in range(2):
                    ds = slice(dc * 512, (dc + 1) * 512)
                    ps = mm_ps.tile([128, 512], F32, tag="mm", name="ps_o")
                    for ch in range(2):
                        nc.tensor.matmul(
                            ps,
                            valst_sb[:, ch, ss],
                            wout_sb[:, ch, ds],
                            start=(ch == 0), stop=(ch == 1))
                    stg = spool.tile([128, 512], F32, tag="stg", name="stg")
                    nc.vector.tensor_copy(stg, ps)
                    nc.sync.dma_start(out=out[ss, ds], in_=stg)

    nc.finalize()
    return nc


def _host_prep(x, mask, W_qkv, b_qkv, W_out):
    """Build the 8 per-core input maps."""
    in_maps = []
    for c in range(N_CORES):
        b = c // 4
        heads = [4 * (c % 4) + j for j in range(NH_LOC)]
        xt = np.ascontiguousarray(x[b].T).astype(np.float32, copy=False)

        qcols = [W_qkv[:, 192 * h:192 * h + 64] for h in heads]
        kcols = [W_qkv[:, 192 * h + 64:192 * h + 128] for h in heads]
        wqk = np.concatenate(qcols + kcols, axis=1).astype(np.float32, copy=False)
        wqk = np.ascontiguousarray(wqk)

        qb = [b_qkv[192 * h:192 * h + 64] for h in heads]
        kb = [b_qkv[192 * h + 64:192 * h + 128] for h in heads]
        bqk = np.stack([
            np.concatenate([qb[0], qb[1]]),
            np.concatenate([qb[2], qb[3]]),
            np.concatenate([kb[0], kb[1]]),
            np.concatenate([kb[2], kb[3]]),
        ], axis=1).astype(np.float32)          # [128, 4]

        wv = np.zeros((D, CV), dtype=np.float32)
        bv_row = np.zeros((CV,), dtype=np.float32)
        for j, h in enumerate(heads):
            wv[:, j * 65:j * 65 + 64] = W_qkv[:, 192 * h + 128:192 * h + 192]
            bv_row[j * 65:j * 65 + 64] = b_qkv[192 * h + 128:192 * h + 192]
            bv_row[j * 65 + 64] = 1.0
        bv = np.broadcast_to(bv_row, (128, CV)).astype(np.float32)
        bv = np.ascontiguousarray(bv)

        wout = np.concatenate([W_out[64 * h:64 * h + 64, :] for h in heads],
                              axis=0).astype(np.float32)
        wout = np.ascontiguousarray(wout)

        maskb = ((mask[b].reshape(ST, 128).T.astype(np.float32)) - 1.0) * 1e9
        maskb = np.ascontiguousarray(maskb)

        in_maps.append({"xt": xt, "wqk": wqk, "bqk": bqk, "wv": wv, "bv": bv,
                        "wout": wout, "maskb": maskb})
    return in_maps


def kernel(x, mask, W_qkv, b_qkv, W_out, b_out):
    x = np.asarray(x, dtype=np.float32)
    mask = np.asarray(mask)
    W_qkv = np.asarray(W_qkv, dtype=np.float32)
    b_qkv = np.asarray(b_qkv, dtype=np.float32)
    W_out = np.asarray(W_out, dtype=np.float32)
    b_out = np.asarray(b_out, dtype=np.float32)

    if "nc" not in _CACHED:
        _CACHED["nc"] = _build_module()
    nc = _CACHED["nc"]

    in_maps = _host_prep(x, mask, W_qkv, b_qkv, W_out)
    trace = os.environ.get("TRNMHA_TRACE") == "1"
    res = run_bass_kernel_spmd(nc, in_maps, core_ids=list(range(N_CORES)),
                               trace=trace)
    if trace and res.exec_time_ns is not None:
        print(f"HW exec time: {res.exec_time_ns} ns")
        if res.instructions_and_trace is not None:
            print(f"trace: {res.instructions_and_trace[1]}")

    out = np.empty((B, S, D), dtype=np.float32)
    for b in range(B):
        acc = res.results[4 * b]["out"].astype(np.float32)
        for c in range(4 * b + 1, 4 * b + 4):
            acc = acc + res.results[c]["out"]
        out[b] = acc + b_out[None, :]
    return out


# revision 11
# speedup vs baseline: 1.1266x; 1.1266x over previous
"""Multi-head attention (B=2, S=2048, D=1024, H=16) on 8 Trainium2 NeuronCores.

Sharding: data-parallel over batch (2 groups of 4 cores) x tensor-parallel over
heads (4 heads per core). Each core:
  - computes qT/kT = (x @ Wqk_c).T via W.T @ x.T   (transposed layout, [ch, S])
  - computes v' = x @ Wv_c augmented with a ones column per head (for sumexp)
  - flash-style attention per head with exp on ScalarE (no max subtraction
    needed: scores ~ N(0,1)); mask folded in as additive bias per k-position
  - partial output projection over its 4 heads' channels -> [S, D]
Host sums the 4 partials per batch and adds b_out.

Projections run in float32r (~1.5e-4 rounding); attention score/AV matmuls in
bf16 (PE 1 cyc/row, 2x faster than f32r). All matmul operands are
base-partition-0 (base-64 fp32-family matmuls hang the HW); per-head K=64
contractions are zero-padded to K=128 instead. Softmax normalization is
deferred: vals/sumexp accumulate unnormalized, reciprocals batch into one DVE
op at the end (a [1,512] DVE reciprocal costs 3.3us, so per-block recips
stalled PE and cold-cycled HAM in v1).
"""
import os
import numpy as np

import concourse.bass as bass
from concourse import bacc
import concourse.mybir as mybir
import concourse.tile as tile
from concourse.bass_utils import run_bass_kernel_spmd

F32 = mybir.dt.float32
F32R = mybir.dt.float32r
BF16 = mybir.dt.bfloat16
U32 = mybir.dt.uint32

B = 2
S = 2048
D = 1024
NH = 16
HD = 64
NH_LOC = 4
N_CORES = 8
KC = D // 128
ST = S // 128
QC = S // 512
CV = NH_LOC * (HD + 1)  # 260

_CACHED = {}


def _build_module():
    nc = bacc.Bacc()
    xt = nc.declare_dram_parameter("xt", [D, S], F32, isOutput=False)
    wqk = nc.declare_dram_parameter("wqk", [D, 512], F32, isOutput=False)
    bqk = nc.declare_dram_parameter("bqk", [128, 4], F32, isOutput=False)
    wv = nc.declare_dram_parameter("wv", [D, CV], F32, isOutput=False)
    bv = nc.declare_dram_parameter("bv", [128, CV], F32, isOutput=False)
    wout = nc.declare_dram_parameter("wout", [2 * 128, D], F32, isOutput=False)
    maskb = nc.declare_dram_parameter("maskb", [128, ST], F32, isOutput=False)
    out = nc.declare_dram_parameter("out", [S, D], F32, isOutput=True)
    recip_dram = nc.dram_tensor("recip_dram", [16, 512], F32)
    sums_dram = nc.dram_tensor("sums_dram", [16, 512], F32)

    with tile.TileContext(nc) as tc:
        with tc.tile_pool(name="persist", bufs=1) as persist, \
             tc.tile_pool(name="epool", bufs=3) as epool, \
             tc.tile_pool(name="spool", bufs=3) as spool, \
             tc.tile_pool(name="mm", bufs=2, space="PSUM") as mm_ps, \
             tc.tile_pool(name="st", bufs=2, space="PSUM") as st_ps, \
             tc.tile_pool(name="vals", bufs=2, space="PSUM") as vals_ps:

            # ---- resident inputs ----
            xt_sb = persist.tile([128, KC, S], F32R)
            wqk_sb = persist.tile([128, KC, 512], F32R)
            wv_sb = persist.tile([128, KC, CV], F32R)
            wout_sb = persist.tile([128, 2, D], F32R)
            bqk_sb = persist.tile([128, 4], F32)
            bv_sb = persist.tile([128, CV], F32)
            maskb_sb = persist.tile([128, ST], F32)
            ones_sb = persist.tile([1, 64], F32R)
            for kc in range(KC):
                nc.sync.dma_start(out=xt_sb[:, kc, :],
                                  in_=xt[kc * 128:(kc + 1) * 128, :].bitcast(F32R))
                nc.sync.dma_start(out=wqk_sb[:, kc, :],
                                  in_=wqk[kc * 128:(kc + 1) * 128, :].bitcast(F32R))
                nc.sync.dma_start(out=wv_sb[:, kc, :],
                                  in_=wv[kc * 128:(kc + 1) * 128, :].bitcast(F32R))
            nc.sync.dma_start(out=wout_sb[:, 0, :], in_=wout[0:128, :].bitcast(F32R))
            nc.sync.dma_start(out=wout_sb[:, 1, :], in_=wout[128:256, :].bitcast(F32R))
            nc.sync.dma_start(out=bqk_sb, in_=bqk[:, :])
            nc.sync.dma_start(out=bv_sb, in_=bv[:, :])
            nc.sync.dma_start(out=maskb_sb, in_=maskb[:, :])
            # 1.0f bit pattern through a uint32 view (direct f32r memset is
            # rejected by walrus codegen)
            nc.vector.memset(ones_sb.bitcast(U32), 0x3F800000)

            # ---- projected tensors (bf16 for the attention matmuls) ----
            qt_sb = persist.tile([128, 2, S], BF16)   # pair hp: rows 0:64=h(2hp), 64:128=h(2hp+1)
            kt_sb = persist.tile([128, 4, S], BF16)   # per-head, zero-padded rows
            v_sb = persist.tile([128, ST, CV], BF16)  # v' natural layout
            valst_sb = persist.tile([128, 2, S], F32R)  # unnormalized attn out (ch x S)
            sums_sb = persist.tile([16, 512], F32)   # sumexp row per (hp,qc,j)
            recip_sb = persist.tile([16, 512], F32R)

            nc.gpsimd.memset(kt_sb[64:128, 0, :].bitcast(mybir.dt.uint16), 0)
            nc.gpsimd.memset(kt_sb[0:64, 1, :].bitcast(mybir.dt.uint16), 0)
            nc.gpsimd.memset(kt_sb[64:128, 2, :].bitcast(mybir.dt.uint16), 0)
            nc.gpsimd.memset(kt_sb[0:64, 3, :].bitcast(mybir.dt.uint16), 0)

            # ---- phase 1: qT/kT projection ----
            for m in (0, 2, 1, 3):
                for qc in range(QC):
                    ps = mm_ps.tile([128, 512], F32, tag="mm", name="ps_qk")
                    for kc in range(KC):
                        nc.tensor.matmul(
                            ps,
                            wqk_sb[:, kc, m * 128:(m + 1) * 128],
                            xt_sb[:, kc, qc * 512:(qc + 1) * 512],
                            start=(kc == 0), stop=(kc == KC - 1))
                    qs = slice(qc * 512, (qc + 1) * 512)
                    if m < 2:
                        nc.vector.tensor_scalar_add(
                            qt_sb[:, m, qs], ps, bqk_sb[:, m:m + 1])
                    else:
                        j0, j1 = 2 * (m - 2), 2 * (m - 2) + 1
                        nc.vector.tensor_scalar_add(
                            kt_sb[0:64, j0, qs], ps[0:64, :], bqk_sb[0:64, m:m + 1])
                        nc.vector.tensor_scalar_add(
                            kt_sb[64:128, j1, qs], ps[64:128, :], bqk_sb[64:128, m:m + 1])

            # ---- phase 2: v' projection ----
            for st in range(ST):
                ps = mm_ps.tile([128, CV], F32, tag="mm", name="ps_v")
                for kc in range(KC):
                    nc.tensor.matmul(
                        ps,
                        xt_sb[:, kc, st * 128:(st + 1) * 128],
                        wv_sb[:, kc, :],
                        start=(kc == 0), stop=(kc == KC - 1))
                nc.vector.tensor_add(v_sb[:, st, :], ps, bv_sb)

            # ---- phase 3: attention (unnormalized) ----
            for hp in range(2):
                j0, j1 = 2 * hp, 2 * hp + 1
                for qc in range(QC):
                    qs = slice(qc * 512, (qc + 1) * 512)
                    va = vals_ps.tile([128, 512], F32, tag="vals", name="va")
                    vb = vals_ps.tile([128, 512], F32, tag="vals", name="vb")
                    for kc in range(ST):
                        ks = slice(kc * 128, (kc + 1) * 128)
                        stp = st_ps.tile([128, 2, 512], F32, tag="st", name="stp")
                        nc.tensor.matmul(stp[:, 0, :], kt_sb[:, j0, ks],
                                         qt_sb[:, hp, qs], start=True, stop=True)
                        nc.tensor.matmul(stp[:, 1, :], kt_sb[:, j1, ks],
                                         qt_sb[:, hp, qs], start=True, stop=True)
                        e_sb = epool.tile([128, 2, 512], BF16, tag="e", name="e_sb")
                        nc.scalar.activation(
                            out=e_sb, in_=stp,
                            func=mybir.ActivationFunctionType.Exp,
                            bias=maskb_sb[:, kc:kc + 1], scale=0.125)
                        nc.tensor.matmul(
                            va[0:65, :], v_sb[:, kc, j0 * 65:j0 * 65 + 65],
                            e_sb[:, 0, :], start=(kc == 0), stop=(kc == ST - 1))
                        nc.tensor.matmul(
                            vb[0:65, :], v_sb[:, kc, j1 * 65:j1 * 65 + 65],
                            e_sb[:, 1, :], start=(kc == 0), stop=(kc == ST - 1))
                    # quick drains: free the vals slots fast, normalize later.
                    # sumexp rows go to DRAM via a same-partition SBUF stage
                    # (DVE cannot write across partitions, DMA cannot read PSUM)
                    for jj, va_ps in ((j0, va), (j1, vb)):
                        row = 8 * hp + 2 * qc + (jj % 2)
                        nc.vector.tensor_copy(
                            valst_sb[(jj % 2) * 64:(jj % 2) * 64 + 64, jj // 2, qs],
                            va_ps[0:64, :])
                        sst = epool.tile([128, 512], F32, tag="sst", name="sst")
                        nc.vector.tensor_copy(sst[64:65, :], va_ps[64:65, :])
                        nc.sync.dma_start(out=sums_dram[row:row + 1, :],
                                          in_=sst[64:65, :])

            # ---- phase 3b: batched reciprocal + deferred normalization ----
            # Broadcast across partitions via a DRAM round-trip (engines cannot
            # partition-broadcast SBUF; DMA from DRAM with step-0 partition can)
            nc.sync.dma_start(out=sums_sb, in_=sums_dram[:, :])
            with nc.allow_low_precision(reason="f32r storage of reciprocal"):
                nc.vector.reciprocal(recip_sb, sums_sb)
            nc.sync.dma_start(out=recip_dram[:, :].bitcast(F32R), in_=recip_sb)
            for hp in range(2):
                for qc in range(QC):
                    qs = slice(qc * 512, (qc + 1) * 512)
                    for jj in (2 * hp, 2 * hp + 1):
                        row = 8 * hp + 2 * qc + (jj % 2)
                        half = slice((jj % 2) * 64, (jj % 2) * 64 + 64)
                        rb = epool.tile([128, 512], F32R, tag="rb", name="rb")
                        nc.sync.dma_start(
                            out=rb[half, :],
                            in_=recip_dram[row:row + 1, :].bitcast(F32R)
                            .to_broadcast([64, 512]))
                        vs = valst_sb[half, jj // 2, qs]
                        # vs = (rb * 1.0) * vs  -- single fused DVE op
                        nc.vector.scalar_tensor_tensor(
                            out=vs, in0=rb[half, :], scalar=1.0, in1=vs,
                            op0=mybir.AluOpType.mult, op1=mybir.AluOpType.mult)

            # ---- phase 4: output projection ----
            for sti in range(ST):
                ss = slice(sti * 128, (sti + 1) * 128)
                for dc in range(2):
                    ds = slice(dc * 512, (dc + 1) * 512)
                    ps = mm_ps.tile([128, 512], F32, tag="mm", name="ps_o")
                    for ch in range(2):
                        nc.tensor.matmul(
                            ps,
                            valst_sb[:, ch, ss],
                            wout_sb[:, ch, ds],
                            start=(ch == 0), stop=(ch == 1))
                    stg = spool.tile([128, 512], F32, tag="stg", name="stg")
                    nc.vector.tensor_copy(stg, ps)
                    nc.sync.dma_start(out=out[ss, ds], in_=stg)

    nc.finalize()
    return nc


def _host_prep(x, mask, W_qkv, b_qkv, W_out):
    in_maps = []
    for c in range(N_CORES):
        b = c // 4
        heads = [4 * (c % 4) + j for j in range(NH_LOC)]
        xt = np.ascontiguousarray(x[b].T).astype(np.float32, copy=False)

        qcols = [W_qkv[:, 192 * h:192 * h + 64] for h in heads]
        kcols = [W_qkv[:, 192 * h + 64:192 * h + 128] for h in heads]
        wqk = np.ascontiguousarray(
            np.concatenate(qcols + kcols, axis=1).astype(np.float32, copy=False))

        qb = [b_qkv[192 * h:192 * h + 64] for h in heads]
        kb = [b_qkv[192 * h + 64:192 * h + 128] for h in heads]
        bqk = np.stack([
            np.concatenate([qb[0], qb[1]]),
            np.concatenate([qb[2], qb[3]]),
            np.concatenate([kb[0], kb[1]]),
            np.concatenate([kb[2], kb[3]]),
        ], axis=1).astype(np.float32)

        wv = np.zeros((D, CV), dtype=np.float32)
        bv_row = np.zeros((CV,), dtype=np.float32)
        for j, h in enumerate(heads):
            wv[:, j * 65:j * 65 + 64] = W_qkv[:, 192 * h + 128:192 * h + 192]
            bv_row[j * 65:j * 65 + 64] = b_qkv[192 * h + 128:192 * h + 192]
            bv_row[j * 65 + 64] = 1.0
        bv = np.ascontiguousarray(
            np.broadcast_to(bv_row, (128, CV)).astype(np.float32))

        wout = np.ascontiguousarray(np.concatenate(
            [W_out[64 * h:64 * h + 64, :] for h in heads], axis=0).astype(np.float32))

        maskb = np.ascontiguousarray(
            ((mask[b].reshape(ST, 128).T.astype(np.float32)) - 1.0) * 1e9)

        in_maps.append({"xt": xt, "wqk": wqk, "bqk": bqk, "wv": wv, "bv": bv,
                        "wout": wout, "maskb": maskb})
    return in_maps


def kernel(x, mask, W_qkv, b_qkv, W_out, b_out):
    x = np.asarray(x, dtype=np.float32)
    mask = np.asarray(mask)
    W_qkv = np.asarray(W_qkv, dtype=np.float32)
    b_qkv = np.asarray(b_qkv, dtype=np.float32)
    W_out = np.asarray(W_out, dtype=np.float32)
    b_out = np.asarray(b_out, dtype=np.float32)

    if "nc" not in _CACHED:
        _CACHED["nc"] = _build_module()
    nc = _CACHED["nc"]

    in_maps = _host_prep(x, mask, W_qkv, b_qkv, W_out)
    trace = os.environ.get("TRNMHA_TRACE") == "1"
    res = run_bass_kernel_spmd(nc, in_maps, core_ids=list(range(N_CORES)),
                               trace=trace)
    if trace and res.exec_time_ns is not None:
        print(f"HW exec time: {res.exec_time_ns} ns")
        if res.instructions_and_trace is not None:
            print(f"trace: {res.instructions_and_trace[1]}")

    out = np.empty((B, S, D), dtype=np.float32)
    for b in range(B):
        acc = res.results[4 * b]["out"].astype(np.float32)
        for c in range(4 * b + 1, 4 * b + 4):
            acc = acc + res.results[c]["out"]
        out[b] = acc + b_out[None, :]
    return out
